# revision 24
# baseline (speedup 1.0000x reference)
"""Trainium2 Bass kernel for ComplexLinearAndLeakyReLU.

Math (per (b, n) token, E=F=256, 3-vectors):
  R = basis(J): rows U, V, nJ built from J          (elementwise over (b,n,e))
  s_j = U_j X0 + V_j X1 + nJ_j X2
  a = U s0 + V s1 ; b = V s0 - U s1 ; c = nJ s2     (elementwise)
  Y[f,i] = sum_e A[f,e] a[e,i] + Bw[f,e] b[e,i] + Cw[f,e] c[e,i]
  d = W @ Y ; out = Y + Relu(-0.8*dot(Y,d)) * d / (|d|^2 + eps)   (VN leaky relu)

Distribution: data-parallel over batch B=16 -> 2 batches per core on 8 cores.
Weights replicated. Host pre-transposes X, J to [b, e, i, n] so every SBUF
tile loads with e on partitions (no on-device transposes needed); the output
[b, f, i, n] layout falls out of the second matmul directly.

Matmuls run as float32r (~1.5e-4 rel err, 4x faster than fp32 on the PE).
"""

import sys

for _p in ("/opt/trn_rl_repo", "/root/.axon_site/_ro/trn_rl_repo"):
    if _p not in sys.path:
        sys.path.insert(0, _p)

import numpy as np

import concourse.bass as bass
import concourse.tile as tile
from concourse import bacc, mybir
from concourse.bass_utils import run_bass_kernel_spmd

F32 = mybir.dt.float32
F32R = mybir.dt.float32r
BF16 = mybir.dt.bfloat16
AF = mybir.ActivationFunctionType

EPS = 1e-6
B, N, E, F = 16, 1024, 256, 256
NCORES = 8
BLOC = B // NCORES          # batches per core
T = 512                     # tokens per super-block
NSB = BLOC * N // T         # super-blocks per core
T3 = 3 * T

_PROGRAM = None


def _bcast3(plane_ap):
    """[128, T] AP -> broadcast view [128, 3, T] (step 0 over components)."""
    return plane_ap.rearrange("p (o t) -> p o t", o=1).broadcast_to([128, 3, T])


def _v3(tile_ap):
    """[128, 3T] AP -> [128, 3, T] view."""
    return tile_ap.rearrange("p (i t) -> p i t", i=3)


def _build_program(repeat=1):
    nc = bacc.Bacc(trn_type="TRN2", target_bir_lowering=False, debug=False)

    Xd = nc.declare_dram_parameter("X", [BLOC, E, 3, N], F32, isOutput=False)
    Jd = nc.declare_dram_parameter("J", [BLOC, E, 3, N], F32, isOutput=False)
    Ad = nc.declare_dram_parameter("At", [E, F], F32R, isOutput=False)
    Bd = nc.declare_dram_parameter("Bt", [E, F], F32R, isOutput=False)
    Cd = nc.declare_dram_parameter("Ct", [E, F], F32R, isOutput=False)
    Bn = nc.declare_dram_parameter("Bn", [E, F], F32R, isOutput=False)
    Wd = nc.declare_dram_parameter("Wt", [F, F], F32R, isOutput=False)
    Od = nc.declare_dram_parameter("out", [BLOC, F, 3, N], F32, isOutput=True)

    vt = nc.vector
    gp = nc.gpsimd
    sc = nc.scalar

    with tile.TileContext(nc) as tc:
        with (
            tc.tile_pool(name="wts", bufs=1) as wpool,
            tc.tile_pool(name="io", bufs=2) as io,
            tc.tile_pool(name="eb", bufs=1) as eb,
            tc.tile_pool(name="sm", bufs=1) as sm,
            tc.tile_pool(name="abc", bufs=2) as abcp,
            tc.tile_pool(name="xt", bufs=1) as xtp,
            tc.tile_pool(name="ot", bufs=1) as otp,
            tc.tile_pool(name="psy", bufs=2, space="PSUM") as psy,
            tc.tile_pool(name="psd", bufs=2, space="PSUM") as psd,
        ):
            # ---- replicated weights: lhsT tiles [e_chunk 128, F] ----
            wabc = []
            for nm, dram in (("A", Ad), ("B", Bd), ("N", Bn), ("C", Cd)):
                per_c = []
                for c in range(2):
                    w = wpool.tile([128, F], F32R, tag=f"w{nm}{c}")
                    nc.sync.dma_start(w[:], dram[128 * c:128 * (c + 1), :])
                    per_c.append(w)
                wabc.append(per_c)
            wW = []
            for c in range(2):
                w = wpool.tile([128, F], F32R, tag=f"wW{c}")
                nc.sync.dma_start(w[:], Wd[128 * c:128 * (c + 1), :])
                wW.append(w)

            for sb in range(NSB * repeat):
                sb = sb % NSB
                b = sb // (N // T)
                n0 = (sb % (N // T)) * T

                trm = [[None, None] for _ in range(5)]  # [term][echunk]

                for c in range(2):
                    e0 = 128 * c
                    # ---- DMA in: [128e, (i, tok)] ----
                    Xt = io.tile([128, T3], F32, tag="X")
                    nc.sync.dma_start(Xt[:], Xd[b, e0:e0 + 128, :, n0:n0 + T])
                    Jt = io.tile([128, T3], F32, tag="J")
                    nc.sync.dma_start(Jt[:], Jd[b, e0:e0 + 128, :, n0:n0 + T])

                    def pl(t, i):  # component plane [128, T]
                        return t[:, i * T:(i + 1) * T]

                    def pla(ap, i):  # plane of an AP
                        return ap[:, i * T:(i + 1) * T]

                    # ---- basis: |J|, nJ ----
                    sqJ = eb.tile([128, T3], F32, tag="sqJ")
                    sc.activation(sqJ[:], Jt[:], AF.Square)
                    q01 = sm.tile([128, T], F32, tag="q01")
                    vt.tensor_add(q01[:], pl(sqJ, 0), pl(sqJ, 1))
                    jsq = sm.tile([128, T], F32, tag="jsq")
                    vt.tensor_add(jsq[:], q01[:], pl(sqJ, 2))
                    rj = sm.tile([128, T], F32, tag="rj")
                    sc.activation(rj[:], jsq[:], AF.Sqrt)
                    rcp_r = sm.tile([128, T], F32, tag="rcp_r")
                    vt.reciprocal_approx_fast(rcp_r[:], rj[:])
                    # basis tile M, 5-plane blocks for wraparound views:
                    # [U0 U1 U2 U0 U1 | V0 V1 V2 - - | n0 n1 n2 n0 n1]
                    M = eb.tile([128, 15 * T], F32, tag="M")
                    nJ = M[:, 10 * T:13 * T]
                    vt.tensor_mul(_v3(nJ), _v3(Jt[:]), _bcast3(rcp_r[:]))

                    # ---- u_z = -(nJ0^2 + nJ1^2) / (nJ2 + eps) ----
                    rr2 = sm.tile([128, T], F32, tag="rr2")
                    vt.tensor_mul(rr2[:], rcp_r[:], rcp_r[:])
                    n01 = sm.tile([128, T], F32, tag="n01")
                    vt.tensor_mul(n01[:], q01[:], rr2[:])
                    mden = sm.tile([128, T], F32, tag="mden")
                    vt.tensor_scalar(mden[:], pla(nJ, 2), -1.0, -EPS,
                                     op0=mybir.AluOpType.mult, op1=mybir.AluOpType.add)
                    rcp2 = sm.tile([128, T], F32, tag="rcp2")
                    vt.reciprocal_approx_fast(rcp2[:], mden[:])
                    uz = sm.tile([128, T], F32, tag="uz")
                    vt.tensor_mul(uz[:], n01[:], rcp2[:])

                    # ---- U = normalize([nJ0, nJ1, uz]) ----
                    squz = sm.tile([128, T], F32, tag="squz")
                    sc.activation(squz[:], uz[:], AF.Square)
                    usq = sm.tile([128, T], F32, tag="usq")
                    vt.tensor_add(usq[:], n01[:], squz[:])
                    ru = sm.tile([128, T], F32, tag="ru")
                    sc.activation(ru[:], usq[:], AF.Sqrt)
                    rcpu = sm.tile([128, T], F32, tag="rcpu")
                    vt.reciprocal_approx_fast(rcpu[:], ru[:])
                    U = M[:, 0:3 * T]
                    vt.tensor_mul(
                        U[:, 0:2 * T].rearrange("p (i t) -> p i t", i=2),
                        nJ[:, 0:2 * T].rearrange("p (i t) -> p i t", i=2),
                        rcpu[:].rearrange("p (o t) -> p o t", o=1)
                            .broadcast_to([128, 2, T]))
                    vt.tensor_mul(pla(U, 2), uz[:], rcpu[:])

                    # ---- V = U x nJ ----
                    V = M[:, 5 * T:8 * T]
                    P = eb.tile([128, T3], F32, tag="P")
                    Q = eb.tile([128, T3], F32, tag="Q")
                    # duplicate U0,U1 and n0,n1 for wraparound views
                    vt.tensor_copy(M[:, 3 * T:5 * T], M[:, 0:2 * T])
                    vt.tensor_copy(M[:, 13 * T:15 * T], M[:, 10 * T:12 * T])
                    # V_i = U_{i+1} n_{i+2} - U_{i+2} n_{i+1}
                    vt.tensor_mul(_v3(P[:]), _v3(M[:, T:4 * T]),
                                  _v3(M[:, 12 * T:15 * T]))
                    vt.tensor_mul(_v3(Q[:]), _v3(M[:, 2 * T:5 * T]),
                                  _v3(M[:, 11 * T:14 * T]))
                    vt.tensor_sub(_v3(V), _v3(P[:]), _v3(Q[:]))

                    # ---- s_j = U_j X0 + V_j X1 + nJ_j X2 ----
                    s = eb.tile([128, T3], F32, tag="s")
                    vt.tensor_mul(_v3(P[:]), _v3(U), _bcast3(pl(Xt, 0)))
                    vt.tensor_mul(_v3(Q[:]), _v3(V), _bcast3(pl(Xt, 1)))
                    vt.tensor_add(_v3(P[:]), _v3(P[:]), _v3(Q[:]))
                    vt.tensor_mul(_v3(Q[:]), _v3(nJ), _bcast3(pl(Xt, 2)))
                    vt.tensor_add(_v3(s[:]), _v3(P[:]), _v3(Q[:]))

                    # ---- a, b, c terms (f32r, feed matmul 1) ----
                    at = abcp.tile([128, T3], F32R, tag="a")
                    bt = abcp.tile([128, T3], F32R, tag="b")
                    ct = abcp.tile([128, T3], F32R, tag="c")
                    M4 = M[:].rearrange("p (m x t) -> p m x t", m=3, x=5)
                    Mc = [M4[:, :, i, :] for i in range(3)]
                    vt.tensor_mul(_v3(P[:]), Mc[0], _bcast3(pl(s, 0)))
                    vt.tensor_mul(_v3(Q[:]), Mc[1], _bcast3(pl(s, 1)))
                    vt.tensor_add(_v3(at[:]), _v3(P[:]), _v3(Q[:]))
                    vt.tensor_mul(_v3(P[:]), Mc[1], _bcast3(pl(s, 0)))
                    vt.tensor_mul(_v3(Q[:]), Mc[0], _bcast3(pl(s, 1)))
                    vt.tensor_sub(_v3(bt[:]), _v3(P[:]), _v3(Q[:]))
                    vt.tensor_mul(_v3(ct[:]), Mc[2], _bcast3(pl(s, 2)))
                    trm[0][c], trm[1][c], trm[2][c] = at, bt, ct

                # ---- matmul 1: Y[f, (i,tok)] = sum_e {A,B,C}.T-contract ----
                x_t = []
                for m in range(2):
                    xm = xtp.tile([128, T3], F32R, tag=f"x{m}")
                    for i in range(3):
                        py = psy.tile([128, T], F32, tag="py")
                        k = 0
                        wmap = [0, 1, 3]  # A, B, C
                        for t_ in range(3):
                            for c in range(2):
                                nc.tensor.matmul(
                                    py[:],
                                    wabc[wmap[t_]][c][:, m * 128:(m + 1) * 128],
                                    trm[t_][c][:, i * T:(i + 1) * T],
                                    start=(k == 0), stop=(k == 5))
                                k += 1
                        sc.activation(xm[:, i * T:(i + 1) * T], py[:], AF.Copy)
                    x_t.append(xm)

                # ---- matmul 2 + VN leaky relu, per output f-chunk ----
                for m in range(2):
                    pd = psd.tile([128, T3], F32, tag="pd")
                    for i in range(3):
                        for c in range(2):
                            nc.tensor.matmul(
                                pd[:, i * T:(i + 1) * T],
                                wW[c][:, m * 128:(m + 1) * 128],
                                x_t[c][:, i * T:(i + 1) * T],
                                start=(c == 0), stop=(c == 1))

                    dsb = eb.tile([128, T3], F32, tag="s")
                    sc.activation(dsb[:], pd[:], AF.Copy)
                    xm = x_t[m][:].bitcast(F32)

                    def xpl(i):
                        return xm[:, i * T:(i + 1) * T]

                    tt = eb.tile([128, T3], F32, tag="P")
                    vt.tensor_mul(_v3(tt[:]), _v3(xm), _v3(dsb[:]))
                    dot = sm.tile([128, T], F32, tag="dot")
                    vt.tensor_reduce(
                        dot[:].rearrange("p (z t) -> p t z", z=1),
                        tt[:].rearrange("p (i t) -> p t i", i=3),
                        axis=mybir.AxisListType.X, op=mybir.AluOpType.add)
                    sqd = eb.tile([128, T3], F32, tag="Q")
                    sc.activation(sqd[:], dsb[:], AF.Square)
                    dn = sm.tile([128, T], F32, tag="dn")
                    vt.tensor_reduce(
                        dn[:].rearrange("p (z t) -> p t z", z=1),
                        sqd[:].rearrange("p (i t) -> p t i", i=3),
                        axis=mybir.AxisListType.X, op=mybir.AluOpType.add)
                    dne = sm.tile([128, T], F32, tag="dne")
                    vt.tensor_scalar_add(dne[:], dn[:], EPS)
                    rcd = sm.tile([128, T], F32, tag="rcd")
                    vt.reciprocal_approx_fast(rcd[:], dne[:])
                    mre = sm.tile([128, T], F32, tag="mre")
                    vt.tensor_scalar(mre[:], dot[:], -0.8, 0.0,
                                     op0=mybir.AluOpType.mult, op1=mybir.AluOpType.max)
                    g = sm.tile([128, T], F32, tag="g")
                    vt.tensor_mul(g[:], mre[:], rcd[:])

                    ot = otp.tile([128, T3], F32, tag=f"o{m}")
                    vt.tensor_mul(_v3(ot[:]), _v3(dsb[:]), _bcast3(g[:]))
                    vt.tensor_add(_v3(ot[:]), _v3(ot[:]), _v3(xm))
                    nc.sync.dma_start(
                        Od[b, m * 128:(m + 1) * 128, :, n0:n0 + T], ot[:])

    nc.finalize()
    return nc


def _get_program():
    global _PROGRAM
    if _PROGRAM is None:
        _PROGRAM = _build_program()
    return _PROGRAM


def kernel(X, J, A, Bw, Cw, W, device=None, **_unused):
    X = np.asarray(X, dtype=np.float32)
    J = np.asarray(J, dtype=np.float32)
    At = np.ascontiguousarray(np.asarray(A, np.float32).T)
    Bt = np.ascontiguousarray(np.asarray(Bw, np.float32).T)
    Ct = np.ascontiguousarray(np.asarray(Cw, np.float32).T)
    Wt = np.ascontiguousarray(np.asarray(W, np.float32).T)    # [F_in, F_out]
    Bnn = np.ascontiguousarray(-np.asarray(Bw, np.float32).T)

    # host relayout: [b, n, e, i] -> [b, e, i, n] per core shard
    in_maps = []
    for d in range(NCORES):
        sl = slice(BLOC * d, BLOC * (d + 1))
        Xs = np.ascontiguousarray(np.transpose(X[sl], (0, 2, 3, 1)))
        Js = np.ascontiguousarray(np.transpose(J[sl], (0, 2, 3, 1)))
        in_maps.append({"X": Xs, "J": Js, "At": At, "Bt": Bt, "Ct": Ct,
                        "Bn": Bnn, "Wt": Wt})

    nc = _get_program()
    res = run_bass_kernel_spmd(nc, in_maps, list(range(NCORES)))

    # gather: per-core out [BLOC, F, 3, N] -> full [B, F, 3, N]
    out = np.concatenate([r["out"] for r in res.results], axis=0)
    return out
